# revision 1
# baseline (speedup 1.0000x reference)
"""Contrastive-loss kernel for 8 TRN2 NeuronCores (Bass/Tile, SPMD).

Math (reference, margin=1):
    d_ij = |x_i|^2 + |x_j|^2 - 2 x_i.x_j            (clamped >= 0)
    pos  = sum_{i!=j, same class} d_ij
    neg  = sum_{i!=j, diff class} relu(1 - sqrt(d_ij))^2
    loss = (pos + neg) / (2 n (n-1))

Device algorithm:
  * Augmented matmul: y_ij = A_i . B_j = d_ij + delta + L * same_ij with
    A_i = [-2 x_i | 1, |x_i|^2, sqrt(delta), lam*onehot_i],
    B_j = [ x_j   | |x_j|^2+?, 1, sqrt(delta), lam*onehot_j],  L = lam^2 = 65536.
    The whole distance matrix *and* the class mask come out of the
    TensorEngine accumulation with no elementwise fixup passes.
  * Feature part (K=512) runs as fp8e4m3 DoubleRow matmuls (2 K-rows per
    PE cell -> 2 matmuls instead of 4); the exact-sensitive tail
    (norms, constants, lam*onehot mask; K rows 512..639, zero padded)
    stays bf16: 3 matmuls per 128-row out tile instead of 5.
  * pos partial sums: relu(y - L) zeroes every different-class entry
    (y < ~2600 << L) and recovers d + delta for same-class entries
    exactly (Sterbenz); ScalarE Relu activation + accum_out reduces for free.
  * neg term: nonzero only if some pair has d < 1, i.e. y < 1 + delta
    (same-class pairs sit at y >= L, never below). VectorE reduce-min of y
    detects this; for randn features min d ~ 660 so neg == 0 exactly.
    If the detector ever fires, the host recomputes the neg term exactly.
  * Symmetry: only block-pairs (a <= b) of 16 row-blocks (512 rows) are
    computed: 136 pairs -> 17 per core via the (k, 15-k) pairing;
    off-diagonal pairs weighted 2x. All cores run the same instruction
    stream; the host routes different block data to each core (SPMD).
"""

import numpy as np
import ml_dtypes

N, C, NCLS = 8192, 512, 100
NB, BS = 16, 512          # row blocks
NPAIR = 17                # block-pairs per core (2 self + 15 off-diagonal)
KC, KP = 5, 640           # bf16 K chunks of 128 (615 used, zero-padded)
LAM = 256.0
L = LAM * LAM             # 65536, exact in fp32/bf16
SDELTA = 0.0625           # sqrt(delta); delta = 2^-8 keeps y > 0
DELTA = SDELTA * SDELTA
MARGIN = 1.0

FP8 = ml_dtypes.float8_e4m3

_CACHE: dict = {}


def _build_bass():
    import concourse.bacc as bacc
    import concourse.mybir as mybir
    import concourse.tile as tile

    nc = bacc.Bacc(
        "TRN2",
        target_bir_lowering=False,
        debug=False,
        enable_asserts=False,
        num_devices=8,
    )
    # fp8 feature part (2048 B) + bf16 tail (1024 B), packed per partition
    lhs_d = nc.dram_tensor(
        "lhs", [NPAIR, 128, 3072], mybir.dt.uint8, kind="ExternalInput"
    ).ap()
    rhs_d = nc.dram_tensor(
        "rhs", [NPAIR, 128, 3072], mybir.dt.uint8, kind="ExternalInput"
    ).ap()
    pacc_d = nc.dram_tensor(
        "pacc", [128, 32], mybir.dt.float32, kind="ExternalOutput"
    ).ap()
    mny_d = nc.dram_tensor(
        "mny", [128, 32], mybir.dt.float32, kind="ExternalOutput"
    ).ap()

    DR = mybir.MatmulPerfMode.DoubleRow

    with tile.TileContext(nc) as tc:
        with (
            tc.tile_pool(name="io", bufs=1) as iop,
            tc.tile_pool(name="rp", bufs=4) as rp,
            tc.tile_pool(name="lp", bufs=4) as lp,
            tc.tile_pool(name="scrp", bufs=2) as scrp,
            tc.tile_pool(name="psp", bufs=2, space="PSUM") as psp,
        ):
            pacc = iop.tile([128, 32], mybir.dt.float32)
            mny = iop.tile([128, 32], mybir.dt.float32)
            negL = iop.tile([128, 1], mybir.dt.float32)
            nc.vector.memset(negL[:], -L)
            nc.vector.memset(pacc[:], 0.0)
            nc.vector.memset(mny[:], 3.0e38)

            for t in range(NPAIR):
                # Alternate which side rides the (slower) SWDGE queue so the
                # late-arrival penalty doesn't always hit the same operand.
                q_rt, q_lt = (nc.sync, nc.gpsimd) if t % 2 == 0 else (nc.gpsimd, nc.sync)
                rt = rp.tile([128, 3072], mybir.dt.uint8)
                q_rt.dma_start(rt[:], rhs_d[t])
                lt = lp.tile([128, 3072], mybir.dt.uint8)
                q_lt.dma_start(lt[:], lhs_d[t])
                rt8 = rt[:, 0:2048].bitcast(mybir.dt.float8e4).rearrange(
                    "p (c i n) -> p c i n", c=2, i=2
                )
                rtb = rt[:, 2048:3072].bitcast(mybir.dt.bfloat16)
                lt8 = lt[:, 0:2048].bitcast(mybir.dt.float8e4).rearrange(
                    "p (c i n) -> p c i n", c=2, i=2
                )
                ltb = lt[:, 2048:3072].bitcast(mybir.dt.bfloat16)

                ps = psp.tile([128, 4 * BS], mybir.dt.float32)
                for r in range(4):
                    out = ps[:, r * BS : (r + 1) * BS]
                    nc.tensor.matmul(
                        out,
                        lt8[:, 0, :, r * 128 : (r + 1) * 128],
                        rt8[:, 0, :, :],
                        start=True,
                        stop=False,
                        perf_mode=DR,
                    )
                    nc.tensor.matmul(
                        out,
                        lt8[:, 1, :, r * 128 : (r + 1) * 128],
                        rt8[:, 1, :, :],
                        start=False,
                        stop=False,
                        perf_mode=DR,
                    )
                    nc.tensor.matmul(
                        out,
                        ltb[:, r * 128 : (r + 1) * 128],
                        rtb,
                        start=False,
                        stop=True,
                    )
                scr = scrp.tile([128, 4 * BS], mybir.dt.bfloat16)
                nc.scalar.activation(
                    scr[:],
                    ps[:],
                    mybir.ActivationFunctionType.Relu,
                    bias=negL[:],
                    scale=1.0,
                    accum_out=pacc[:, t : t + 1],
                )
                nc.vector.tensor_reduce(
                    mny[:, t : t + 1],
                    ps[:],
                    axis=mybir.AxisListType.X,
                    op=mybir.AluOpType.min,
                )

            nc.sync.dma_start(pacc_d[:], pacc[:])
            nc.sync.dma_start(mny_d[:], mny[:])

    nc.compile()
    return nc


def _pair_lists():
    """Per-core block-pair assignment covering every unordered pair once."""
    cores = []
    for k in range(8):
        pairs = [(k, k), (15 - k, 15 - k)]
        pairs += [(k, b) for b in range(k + 1, 16)]
        pairs += [(15 - k, b) for b in range(16 - k, 16)]
        assert len(pairs) == NPAIR
        cores.append(pairs)
    return cores


def _prep_blocks(features: np.ndarray, target: np.ndarray):
    """Per-block operand arrays.

    Returns (A8, B8, Ab, Bb):
      A8/B8: [16, 128, 2, 2, 512] fp8  — feature part, DoubleRow layout;
             K-row 256c+128i+p lives at [blk, p, c, i, m].
      Ab/Bb: [16, 128, 512] bf16       — tail chunk (K rows 512..639).
    """
    f = np.ascontiguousarray(features, np.float32)
    sq = np.einsum("ij,ij->i", f, f, dtype=np.float32).astype(np.float32)
    oh = np.zeros((N, NCLS), np.float32)
    oh[np.arange(N), target.astype(np.int64)] = LAM

    TK = KP - C  # 128 tail rows
    At = np.zeros((N, TK), np.float32)
    Bt = np.zeros((N, TK), np.float32)
    At[:, 0] = 1.0
    At[:, 1] = sq
    At[:, 2] = SDELTA
    At[:, 3 : 3 + NCLS] = oh
    Bt[:, 0] = sq
    Bt[:, 1] = 1.0
    Bt[:, 2] = SDELTA
    Bt[:, 3 : 3 + NCLS] = oh

    def feat8(M):  # [N, C] f32 -> [16, 128, 2, 2, BS] fp8
        X = M.astype(FP8).reshape(NB, BS, 2, 2, 128)  # [blk, m, c, i, p]
        return np.ascontiguousarray(X.transpose(0, 4, 2, 3, 1))

    def tailb(M):  # [N, TK] f32 -> [16, 128, BS] bf16
        X = M.astype(ml_dtypes.bfloat16).reshape(NB, BS, TK)  # [blk, m, k]
        return np.ascontiguousarray(X.transpose(0, 2, 1))

    def pack(f8, fb):  # -> [16, 128, 3072] uint8
        return np.concatenate(
            [
                f8.view(np.uint8).reshape(NB, 128, 2048),
                fb.view(np.uint8).reshape(NB, 128, 1024),
            ],
            axis=-1,
        )

    return (
        pack(feat8(-2.0 * f), tailb(At)),
        pack(feat8(f), tailb(Bt)),
    )


def _make_in_maps(features: np.ndarray, target: np.ndarray):
    Apk, Bpk = _prep_blocks(features, target)
    in_maps = []
    for pairs in _pair_lists():
        ai = [a for a, _ in pairs]
        bi = [b for _, b in pairs]
        in_maps.append(
            {
                "lhs": np.ascontiguousarray(Apk[ai]),
                "rhs": np.ascontiguousarray(Bpk[bi]),
            }
        )
    return in_maps


def _host_neg_term(features: np.ndarray, target: np.ndarray) -> float:
    """Exact fp32 recompute of the negative (hinge) term, mirroring the
    reference elementwise ops. Only runs if the on-device detector finds
    any pair with d < ~margin^2 (never, for randn features)."""
    f = np.asarray(features, np.float32)
    sq = (f * f).sum(1)
    d = sq[:, None] + sq[None, :] - 2.0 * (f @ f.T)
    d = np.maximum(d, 0.0)
    tg = np.asarray(target)
    same = tg[:, None] == tg[None, :]
    eye = np.eye(N, dtype=bool)
    neg_mask = (~same) & (~eye)
    tmp = np.where(d > 0, MARGIN - np.sqrt(np.where(d > 0, d, 1.0)), MARGIN)
    neg = np.where(neg_mask & (tmp > 0), tmp, 0.0)
    return float((neg.astype(np.float64) ** 2).sum())


def kernel(features, target):
    from concourse import bass_utils

    features = np.asarray(features, np.float32)
    target = np.asarray(target)
    assert features.shape == (N, C)

    if "nc" not in _CACHE:
        _CACHE["nc"] = _build_bass()
    nc = _CACHE["nc"]

    in_maps = _make_in_maps(features, target)
    res = bass_utils.run_bass_kernel_spmd(nc, in_maps, core_ids=list(range(8)))

    pos = 0.0
    min_y = np.inf
    w = np.array([1.0, 1.0] + [2.0] * 15)
    for core_out in res.results:
        pacc = np.asarray(core_out["pacc"], np.float64)[:, :NPAIR]
        mny = np.asarray(core_out["mny"], np.float32)[:, :NPAIR]
        pos += float((pacc.sum(axis=0) * w).sum())
        min_y = min(min_y, float(mny.min()))

    # delta bias correction: every same-class (incl. diagonal) pair gained
    # +delta inside relu(y - L). Counted exactly from the targets.
    _, cnt = np.unique(target, return_counts=True)
    n_same = int((cnt.astype(np.int64) ** 2).sum())
    pos -= DELTA * n_same

    neg = 0.0
    if min_y < 16.0:  # conservative: hinge needs y < 1 + delta; fp8 err << 16
        neg = _host_neg_term(features, target)

    t = N * (N - 1)
    return np.asarray((pos + neg) / (2.0 * t), dtype=np.float32)



# revision 11
# speedup vs baseline: 1.3105x; 1.3105x over previous
"""Contrastive-loss kernel for 8 TRN2 NeuronCores (Bass/Tile, SPMD).

Math (reference, margin=1):
    d_ij = |x_i|^2 + |x_j|^2 - 2 x_i.x_j            (clamped >= 0)
    pos  = sum_{i!=j, same class} d_ij
    neg  = sum_{i!=j, diff class} relu(1 - sqrt(d_ij))^2
    loss = (pos + neg) / (2 n (n-1))

Algorithm:
  * pos via the exact class-sum identity
        sum_{i,j in c} d_ij = 2 n_c S2_c - 2 |S1_c|^2
    (diagonal terms are 0, so the i!=j sum equals the full sum).
    S1_c = sum_{i in c} x_i is computed ON DEVICE with small bf16
    matmuls (onehot^T @ X per 128-row chunk, accumulated in PSUM);
    S2_c / n_c are O(N) host reductions.
  * neg is zero iff every different-class pair has d >= 1.  The device
    certifies this: it computes the full Gram matrix G = X X^T (fp8
    DoubleRow matmuls, block-pair tiled, each unordered block-pair
    once) and reduces each [128,512] tile to a max (or a
    relu-threshold detector on the Scalar engine).  Host check:
        min d >= min_row |x_i|^2 + min_col |x_j|^2 - 2 (maxG + FP8ERR)
    with exact host-side min-norms and a rigorous fp8 error bound.
    If any tile fails the bound (never, for real data), the host
    recomputes neg exactly.
  * Self block-pairs (a==b) would have the max dominated by the
    diagonal G_ii = |x_i|^2, so they get one extra bf16 matmul adding
    -lam^2 * same_class(i,j) (lam=256) which pushes all same-class
    entries (incl. the diagonal) to ~-65536, leaving the max over
    different-class pairs only.
  * Block-pair coverage: the 136 unordered pairs of 16 row-blocks are
    oriented by the circulant tournament (edge {a,b} belongs to a iff
    (b-a) mod 16 <= 7, diameter edges to the low block).  Core k owns
    blocks {k, 15-k}: exactly 9 pairs for block k (partners k..k+8
    mod 16) and 8 for block 15-k -- a fixed SPMD template, with the
    self pairs at t=0 and t=9.
  * Reductions are split DVE (max over free axis) / Scalar (relu
    detector, threshold 250) so both stay under the TensorE pipeline
    (GpSimd cannot read PSUM on TRN2).  A few dummy bf16 matmuls at
    t=0 keep the PE busy during the first DMAs so the HAM clock-gate
    warms up early.
"""

import numpy as np
import ml_dtypes

N, C, NCLS = 8192, 512, 100
NB, BS = 16, 512          # row blocks
NPAIR = 17                # block-pairs per core (t=0 / t=9 are self pairs)
NUNIT = NPAIR * 4         # [128, 512] output tiles per core
LAM = 256.0
MARGIN = 1.0
THRESH = 250.0            # scalar-engine relu detector threshold on G
# rigorous |G_fp8 - G_true| bound: per-term rel err <= 2*2^-4 + 2^-8,
# sum_k |x_ik||x_jk| <= |x_i||x_j| <= max_sq (~660 for randn; the host
# check recomputes this bound from the actual data)
FP8_RELERR = 0.13

FP8 = ml_dtypes.float8_e4m3
BF16 = ml_dtypes.bfloat16

_CACHE: dict = {}


def _red_engine(u: int) -> str:
    # V=DVE max-reduce, A=Scalar relu detector (GpSimd cannot read
    # PSUM on TRN2).  Scalar is kept out of the first two pairs: its
    # strict-FIFO queue drains the b-side DMA issues and the act-table
    # load first.
    if u < 8:
        return "V"
    return "V" if u % 9 % 2 == 0 else "A"


def _build_bass():
    import concourse.bacc as bacc
    import concourse.mybir as mybir
    import concourse.tile as tile

    nc = bacc.Bacc(
        "TRN2",
        target_bir_lowering=False,
        debug=False,
        enable_asserts=False,
        num_devices=8,
    )
    ablk_d = nc.dram_tensor(
        "ablk", [2, 128, 2048], mybir.dt.uint8, kind="ExternalInput"
    ).ap()
    bblk_d = nc.dram_tensor(
        "bblk", [NPAIR, 128, 2048], mybir.dt.uint8, kind="ExternalInput"
    ).ap()
    # X in [row-in-chunk, chunk, chan] layout for the S1 contraction (bf16)
    xga_d = nc.dram_tensor(
        "xga", [2, 128, 4096], mybir.dt.uint8, kind="ExternalInput"
    ).ap()
    # onehot (1.0) of the rows, [row-in-chunk, chunk, class] (bf16)
    oha_d = nc.dram_tensor(
        "oha", [2, 128, 800], mybir.dt.uint8, kind="ExternalInput"
    ).ap()
    # +lam*onehot / -lam*onehot per self block, [class, row] (bf16)
    ohp_d = nc.dram_tensor(
        "ohp", [2, 100, 1024], mybir.dt.uint8, kind="ExternalInput"
    ).ap()
    ohm_d = nc.dram_tensor(
        "ohm", [2, 100, 1024], mybir.dt.uint8, kind="ExternalInput"
    ).ap()
    mx_d = nc.dram_tensor(
        "mx", [128, NUNIT], mybir.dt.float32, kind="ExternalOutput"
    ).ap()
    acc_d = nc.dram_tensor(
        "acc", [128, NUNIT], mybir.dt.float32, kind="ExternalOutput"
    ).ap()
    s1_d = nc.dram_tensor(
        "s1", [100, 512], mybir.dt.float32, kind="ExternalOutput"
    ).ap()

    DR = mybir.MatmulPerfMode.DoubleRow

    with tile.TileContext(nc) as tc:
        with (
            tc.tile_pool(name="io", bufs=1) as iop,
            tc.tile_pool(name="psp", bufs=5, space="PSUM") as psp,
            tc.tile_pool(name="ps1", bufs=1, space="PSUM") as ps1p,
            tc.tile_pool(name="psw", bufs=1, space="PSUM") as pswp,
        ):
            mx = iop.tile([128, NUNIT], mybir.dt.float32)
            nc.gpsimd.memset(mx[:], -3.0e38)
            acc = iop.tile([128, NUNIT], mybir.dt.float32)
            nc.gpsimd.memset(acc[:], 0.0)
            scr = iop.tile([128, 512], mybir.dt.bfloat16)
            thr = iop.tile([128, 1], mybir.dt.float32)
            nc.gpsimd.memset(thr[:], -THRESH)

            # --- warmup: keep PE busy while the first blocks DMA in ---
            warm = iop.tile([128, 512], mybir.dt.bfloat16)
            nc.gpsimd.memset(warm[:], 0.0)
            psw = pswp.tile([128, 512], mybir.dt.float32)
            for w in range(8):
                nc.tensor.matmul(
                    psw[:], warm[:, 0:128], warm[:],
                    start=(w == 0), stop=(w == 7),
                )

            # --- input DMAs (all issued upfront; strict-FIFO engine
            # queues mean a consumer emitted before its DMA issue on the
            # same queue would deadlock) ---
            at = iop.tile([128, 2, 2048], mybir.dt.uint8)
            bt = iop.tile([128, NPAIR, 2048], mybir.dt.uint8)
            # b-side in growing chunks on the scalar queue (the first
            # chunks small so pair 0 starts early)
            bchunks = [(0, 1), (1, 2), (2, 4), (4, 7), (7, 10), (10, 13), (13, 17)]
            for lo, hi in bchunks:
                nc.scalar.dma_start(bt[:, lo:hi, :], bblk_d[lo:hi])
            # a-side + masks (small, needed by pair 0/9) then S1 operands
            # on the sync queue
            nc.sync.dma_start(at[:, 0, :], ablk_d[0])
            ohp, ohm = [], []
            for i in range(2):
                p = iop.tile([100, 1024], mybir.dt.uint8)
                nc.sync.dma_start(p[:], ohp_d[i])
                ohp.append(p)
                m = iop.tile([100, 1024], mybir.dt.uint8)
                nc.sync.dma_start(m[:], ohm_d[i])
                ohm.append(m)
            nc.sync.dma_start(at[:, 1, :], ablk_d[1])
            xga, oha = [], []
            for i in range(2):
                x = iop.tile([128, 4096], mybir.dt.uint8)
                nc.sync.dma_start(x[:], xga_d[i])
                xga.append(x)
                o = iop.tile([128, 800], mybir.dt.uint8)
                nc.sync.dma_start(o[:], oha_d[i])
                oha.append(o)

            a8 = at[:].bitcast(mybir.dt.float8e4).rearrange(
                "p t (c i n) -> p t c i n", c=2, i=2
            )
            b8 = bt[:].bitcast(mybir.dt.float8e4).rearrange(
                "p t (c i n) -> p t c i n", c=2, i=2
            )

            def pair(t):
                ai = 0 if t < 9 else 1
                for rt in range(4):
                    u = t * 4 + rt
                    ps = psp.tile([128, 512], mybir.dt.float32)
                    sl = slice(rt * 128, (rt + 1) * 128)
                    nc.tensor.matmul(
                        ps[:], a8[:, ai, 0, :, sl], b8[:, t, 0, :, :],
                        start=True, stop=False, perf_mode=DR,
                    )
                    self_pair = t in (0, 9)
                    nc.tensor.matmul(
                        ps[:], a8[:, ai, 1, :, sl], b8[:, t, 1, :, :],
                        start=False, stop=not self_pair, perf_mode=DR,
                    )
                    if self_pair:
                        pv = ohp[ai][:].bitcast(mybir.dt.bfloat16)
                        mv = ohm[ai][:].bitcast(mybir.dt.bfloat16)
                        nc.tensor.matmul(
                            ps[:], pv[:, sl], mv[:],
                            start=False, stop=True,
                        )
                    eng = _red_engine(u)
                    if eng == "V":
                        nc.vector.tensor_reduce(
                            mx[:, u : u + 1], ps[:],
                            axis=mybir.AxisListType.X, op=mybir.AluOpType.max,
                        )
                    else:
                        nc.scalar.activation(
                            scr[:], ps[:],
                            mybir.ActivationFunctionType.Relu,
                            bias=thr[:], scale=1.0,
                            accum_out=acc[:, u : u + 1],
                        )

            pair(0)
            pair(1)

            # --- S1 partial class sums over this core's two A blocks ---
            pss1 = ps1p.tile([128, 512], mybir.dt.float32)
            for i in range(2):
                xv = xga[i][:].bitcast(mybir.dt.bfloat16).rearrange(
                    "p (h n) -> p h n", h=4
                )
                ov = oha[i][:].bitcast(mybir.dt.bfloat16).rearrange(
                    "p (h m) -> p h m", h=4
                )
                for h in range(4):
                    nc.tensor.matmul(
                        pss1[0:100, :], ov[:, h, :], xv[:, h, :],
                        start=(i == 0 and h == 0), stop=(i == 1 and h == 3),
                    )

            for t in range(2, NPAIR):
                pair(t)

            s1sb = iop.tile([128, 512], mybir.dt.float32)
            nc.vector.tensor_copy(s1sb[0:100, :], pss1[0:100, :])

            nc.sync.dma_start(mx_d[:], mx[:])
            nc.sync.dma_start(acc_d[:], acc[:])
            nc.sync.dma_start(s1_d[:], s1sb[0:100, :])

    nc.compile()
    return nc


def _pair_lists():
    """Per-core block-pair template from the circulant tournament."""
    cores = []
    for k in range(8):
        a0, a1 = k, 15 - k
        pairs = [(a0, (a0 + t) % 16) for t in range(9)]
        pairs += [(a1, (a1 + t) % 16) for t in range(8)]
        assert len(pairs) == NPAIR
        cores.append(pairs)
    # every unordered pair covered exactly once
    seen = set()
    for pairs in cores:
        for a, b in pairs:
            key = (min(a, b), max(a, b))
            assert key not in seen
            seen.add(key)
    assert len(seen) == 136
    return cores


def _prep(features: np.ndarray, target: np.ndarray):
    f = np.ascontiguousarray(features, np.float32)
    tg = np.asarray(target).astype(np.int64)

    # fp8 feature blocks, DoubleRow layout: chan = 256c + 128i + p, col = row
    X8 = f.astype(FP8).reshape(NB, BS, 2, 2, 128)      # [blk, m, c, i, p]
    F8 = np.ascontiguousarray(X8.transpose(0, 4, 2, 3, 1))  # [blk, p, c, i, m]
    F8 = F8.reshape(NB, 128, 2048).view(np.uint8)

    # bf16 X in [row-in-chunk(p), chunk, chan] layout per block
    XG = np.ascontiguousarray(
        f.reshape(NB, 4, 128, C).transpose(0, 2, 1, 3).astype(BF16)
    )  # [blk, 128, 4, 512] bf16
    XG = XG.view(np.uint8).reshape(NB, 128, 4096)

    # onehot(1.0) of rows, [row-in-chunk, chunk, class]
    OH = np.zeros((N, NCLS), np.float32)
    OH[np.arange(N), tg] = 1.0
    OHA = np.ascontiguousarray(
        OH.reshape(NB, 4, 128, NCLS).transpose(0, 2, 1, 3).astype(BF16)
    )
    OHA = OHA.view(np.uint8).reshape(NB, 128, 800)

    # +-lam*onehot, [class, row] per block
    OHT = np.zeros((NB, NCLS, BS), np.float32)
    for blk in range(NB):
        OHT[blk, tg[blk * BS : (blk + 1) * BS], np.arange(BS)] = LAM
    OHP = np.ascontiguousarray(OHT.astype(BF16)).view(np.uint8).reshape(
        NB, NCLS, 1024
    )
    OHM = np.ascontiguousarray((-OHT).astype(BF16)).view(np.uint8).reshape(
        NB, NCLS, 1024
    )
    return F8, XG, OHA, OHP, OHM


def _make_in_maps(features: np.ndarray, target: np.ndarray):
    F8, XG, OHA, OHP, OHM = _prep(features, target)
    in_maps = []
    for k, pairs in enumerate(_pair_lists()):
        bi = [b for _, b in pairs]
        sb = [k, 15 - k]  # this core's A blocks
        in_maps.append(
            {
                "ablk": np.ascontiguousarray(F8[sb]),
                "bblk": np.ascontiguousarray(F8[bi]),
                "xga": np.ascontiguousarray(XG[sb]),
                "oha": np.ascontiguousarray(OHA[sb]),
                "ohp": np.ascontiguousarray(OHP[sb]),
                "ohm": np.ascontiguousarray(OHM[sb]),
            }
        )
    return in_maps


def _host_neg_term(features: np.ndarray, target: np.ndarray) -> float:
    """Exact recompute of the negative (hinge) term; only runs if the
    on-device distance certificate fails (never, for real data)."""
    f = np.asarray(features, np.float32)
    sq = (f * f).sum(1)
    d = sq[:, None] + sq[None, :] - 2.0 * (f @ f.T)
    d = np.maximum(d, 0.0)
    tg = np.asarray(target)
    same = tg[:, None] == tg[None, :]
    eye = np.eye(N, dtype=bool)
    neg_mask = (~same) & (~eye)
    tmp = np.where(d > 0, MARGIN - np.sqrt(np.where(d > 0, d, 1.0)), MARGIN)
    neg = np.where(neg_mask & (tmp > 0), tmp, 0.0)
    return float((neg.astype(np.float64) ** 2).sum())


def kernel(features, target):
    from concourse import bass_utils

    features = np.asarray(features, np.float32)
    target = np.asarray(target)
    assert features.shape == (N, C)

    if "nc" not in _CACHE:
        _CACHE["nc"] = _build_bass()
    nc = _CACHE["nc"]

    in_maps = _make_in_maps(features, target)
    res = bass_utils.run_bass_kernel_spmd(nc, in_maps, core_ids=list(range(8)))

    tg = target.astype(np.int64)
    f64 = features.astype(np.float64)
    sq = np.einsum("ij,ij->i", f64, f64)

    # pos from the class-sum identity (S1 partials from the device)
    S1 = np.zeros((100, 512), np.float64)
    for core_out in res.results:
        S1 += np.asarray(core_out["s1"], np.float64)
    cnt = np.bincount(tg, minlength=NCLS).astype(np.float64)
    S2 = np.zeros(NCLS, np.float64)
    np.add.at(S2, tg, sq)
    pos = float((2.0 * cnt * S2).sum() - 2.0 * (S1 * S1).sum())

    # distance certificate: per [128,512] tile,
    #   min d >= minsq(row band) + minsq(col block) - 2 (maxG + err)
    band_min = sq.reshape(64, 128).min(1)
    blk_min = sq.reshape(NB, BS).min(1)
    err = FP8_RELERR * sq.max()
    ok = True
    for k, pairs in enumerate(_pair_lists()):
        mxv = np.asarray(res.results[k]["mx"], np.float32).max(axis=0)
        accv = np.asarray(res.results[k]["acc"], np.float64).sum(axis=0)
        for t, (a, b) in enumerate(pairs):
            for rt in range(4):
                u = t * 4 + rt
                eng = _red_engine(u)
                if eng == "A":
                    g = THRESH if accv[u] == 0.0 else np.inf
                else:
                    g = float(mxv[u])
                bound = band_min[a * 4 + rt] + blk_min[b] - 2.0 * (g + err)
                if bound < MARGIN * MARGIN:
                    ok = False
    neg = 0.0 if ok else _host_neg_term(features, target)

    t = N * (N - 1)
    return np.asarray((pos + neg) / (2.0 * t), dtype=np.float32)
